# revision 1
# baseline (speedup 1.0000x reference)
"""Associative-embedding loss on 8 Trainium2 NeuronCores.

Data-parallel over batch N=32: each of the 8 cores handles 4 images.
Layout: 128 SBUF partitions = 4 images x 32 rows; rows 0..29 of each
32-block are that image's persons (M=30), rows 30..31 are zero pads
(32-alignment is required by PE tile positions).

Per core the Bass kernel:
  1. DMAs the (int32-cast, padded) joints tensor into SBUF.
  2. Fetches the 4*32*17 tag values with 17 indirect (gather) DMAs —
     one per joint column; walrus's indirect-DMA contract is one index
     and one contiguous run per partition, so a [128,1] dest per DMA is
     the per-element-gather granularity. ~8.7KB of HBM traffic instead
     of streaming the 17.8MB per-core tags slab.
  3. Per-person mean/pull terms on DVE; the 30x30 pairwise push matrix
     via tiny K=1 PE matmuls (rank-1 expansion of (mi-mj)^2 plus a
     large additive term that kills invalid columns under exp); exp on
     ACT with fused row sums; per-image segment sums via one PE matmul
     against a segment-indicator matrix.
  4. Writes per-image (pull_i, push_i) pairs [4,2] to DRAM.
Host concatenates the 8 x [4,2] outputs and takes the mean over all 32
images (the "all-reduce of the final means").
"""

import numpy as np
from contextlib import ExitStack

import concourse.bass as bass
import concourse.tile as tile
from concourse import mybir
from concourse.bass_utils import run_bass_kernel_spmd

# Problem constants (hardcoded per contract).
N, K, H, W, M = 32, 17, 256, 256, 30
NCORES = 8
NLOC = N // NCORES          # images per core
KHW = K * H * W             # 1114112 flat tag elements per image
MP = 32                     # padded persons per image (PE alignment)
P = NLOC * MP               # 128 partitions
BIG = 30.0                  # exp(-BIG) ~ 9e-14: masks invalid columns

f32 = mybir.dt.float32
i32 = mybir.dt.int32
Alu = mybir.AluOpType


def build_nc(debug: bool = False) -> bass.Bass:
    nc = bass.Bass()
    tags_d = nc.declare_dram_parameter("tags", [NLOC, KHW], f32, isOutput=False)
    jt_d = nc.declare_dram_parameter("jt", [P, K, 2], i32, isOutput=False)
    out_d = nc.declare_dram_parameter("out", [NLOC, 2], f32, isOutput=True)
    dbg = {}
    if debug:
        dbg["f"] = nc.declare_dram_parameter("dbgf", [P, 24], f32, isOutput=True)
        dbg["i"] = nc.declare_dram_parameter("dbgi", [P, K], i32, isOutput=True)

    with tile.TileContext(nc) as tc:
        with ExitStack() as ctx:
            _body(ctx, tc, nc, tags_d[:], jt_d[:], out_d[:], dbg)
    _split_multi_waits(nc, max_waits=1)
    return nc


def _split_multi_waits(nc, max_waits=1):
    """Walrus codegen rejects instructions with too many sync-wait commands
    ("Too many sync wait commands", CoreV3GenImpl::setupSyncWait). Tile's
    kernel-tail drain waits on every live semaphore (7 here). Split the
    excess waits onto same-engine nops inserted immediately before the
    offending instruction — identical semantics, one wait per instruction."""
    import bass_rust
    fn = nc.m.functions[0]
    for bb in fn.blocks:
        changed = True
        while changed:
            changed = False
            for inst in list(bb.instructions):
                si = inst.sync_info
                if si is None or not si.on_wait or len(si.on_wait) <= max_waits:
                    continue
                waits = list(si.on_wait)
                keep, rest = waits[:max_waits], waits[max_waits:]
                nops = []
                for i in range(0, len(rest), max_waits):
                    nop_inst = nc.engines[inst.engine].nop().ins
                    nop_inst.sync_info = bass_rust.SyncInfo(
                        on_wait=rest[i:i + max_waits], on_update=[])
                    nops.append(nop_inst)
                inst.sync_info = bass_rust.SyncInfo(
                    on_wait=keep, on_update=list(si.on_update))
                # nop() appended the nops somewhere; move them just before inst
                for b2 in fn.blocks:
                    lst = b2.instructions
                    for i in range(len(lst) - 1, -1, -1):
                        if any(lst[i].name == n.name for n in nops):
                            del lst[i]
                idx = next(i for i, x in enumerate(bb.instructions)
                           if x.name == inst.name)
                for j, n in enumerate(nops):
                    bb.instructions.insert(idx + j, n)
                changed = True
                break


def _body(ctx, tc, nc, tags, jt, out, dbg=None):
    pool = ctx.enter_context(tc.tile_pool(name="main", bufs=1))
    psum = ctx.enter_context(tc.tile_pool(name="psum", bufs=1, space="PSUM"))

    # ---- one-time constants -------------------------------------------------
    seg = pool.tile([P, NLOC], f32)          # seg[p, i] = 1 iff p in image i's
    nc.vector.memset(seg, 0.0)               # first 30 rows
    for ni in range(NLOC):
        nc.vector.memset(seg[ni * MP:ni * MP + M, ni:ni + 1], 1.0)


    # identity for PE transpose; final producer must be DVE so matmuls
    # need only a single (DVE) sync wait — walrus caps LDWEIGHTS waits.
    ii = pool.tile([P, P], f32)
    nc.gpsimd.iota(ii[:], pattern=[[1, P]], base=0, channel_multiplier=-1,
                   allow_small_or_imprecise_dtypes=True)   # ii[p,j] = j - p
    idm = pool.tile([P, P], f32)
    nc.vector.tensor_scalar(out=idm, in0=ii, scalar1=0.0, scalar2=None,
                            op0=Alu.is_equal)

    # ---- load joints, build indices & visibility ---------------------------
    jt3 = pool.tile([P, K, 2], i32)
    nc.sync.dma_start(out=jt3, in_=jt)

    visf = pool.tile([P, K], f32)
    nc.vector.tensor_copy(out=visf, in_=jt3[:, :, 1])   # int32 -> f32 cast
    cnt = pool.tile([P, 1], f32)
    nc.vector.tensor_scalar(
        out=visf, in0=visf, scalar1=0.0, scalar2=None, op0=Alu.is_gt,
    )
    nc.vector.reduce_sum(out=cnt, in_=visf, axis=mybir.AxisListType.X)

    # ---- gather tag values: g[p, k] = tags.flat[idxall[p, k]] --------------
    # HW-verified walrus semantics for indirect gather: one descriptor per
    # partition, reading a CONTIGUOUS run of (dest free size) elements from
    # flat[idx[p, 0]] — per-element gathers therefore need a [P, 1] dest.
    # One indirect DMA per joint column k.
    # (loc channel already holds the absolute flat index — host pre-adds
    # each image's ni*KHW offset while sharding)
    gt = pool.tile([P, K], f32)
    g = gt[:, :]
    for kk in range(K):
        nc.gpsimd.indirect_dma_start(
            out=gt[:, kk:kk + 1], out_offset=None, in_=tags,
            in_offset=bass.IndirectOffsetOnAxis(ap=jt3[:, kk, 0:1], axis=1),
        )

    # ---- per-person stats ---------------------------------------------------
    gv = pool.tile([P, K], f32)
    sumg = pool.tile([P, 1], f32)
    nc.vector.tensor_tensor(out=gv, in0=g, in1=visf, op=Alu.mult)
    nc.vector.reduce_sum(out=sumg, in_=gv, axis=mybir.AxisListType.X)
    safecnt = pool.tile([P, 1], f32)
    nc.vector.tensor_scalar(out=safecnt, in0=cnt, scalar1=1.0, scalar2=None,
                            op0=Alu.max)
    rc = pool.tile([P, 1], f32)
    nc.vector.reciprocal(out=rc, in_=safecnt)

    mrow = pool.tile([P, 4], f32)   # cols: mean, -2*mean, mean^2,
                                    #       mean^2 + BIG*(1-v)
    red = pool.tile([P, 3], f32)    # cols: pull*v, v, rowpush
    mean = mrow[:, 0:1]
    valid = red[:, 1:2]
    nc.vector.tensor_tensor(out=mean, in0=sumg, in1=rc, op=Alu.mult)
    nc.vector.tensor_scalar(out=valid, in0=cnt, scalar1=0.0, scalar2=None,
                            op0=Alu.is_gt)

    # pull: sum_k vis*(g-mean)^2 / safecnt, gated by person validity
    d = pool.tile([P, K], f32)
    nc.vector.tensor_scalar(out=d, in0=g, scalar1=mean, scalar2=None,
                            op0=Alu.subtract)
    dv = pool.tile([P, K], f32)
    nc.vector.tensor_tensor(out=dv, in0=d, in1=visf, op=Alu.mult)
    d2v = pool.tile([P, K], f32)
    pulls = pool.tile([P, 1], f32)
    nc.vector.tensor_tensor(out=d2v, in0=dv, in1=d, op=Alu.mult)
    nc.vector.reduce_sum(out=pulls, in_=d2v, axis=mybir.AxisListType.X)
    nc.vector.scalar_tensor_tensor(out=red[:, 0:1], in0=pulls, scalar=rc[:, 0:1],
                                   in1=valid, op0=Alu.mult, op1=Alu.mult)

    # push prep columns
    nc.vector.tensor_scalar(out=mrow[:, 1:2], in0=mean, scalar1=-2.0,
                            scalar2=None, op0=Alu.mult)
    nc.vector.tensor_tensor(out=mrow[:, 2:3], in0=mean, in1=mean, op=Alu.mult)
    bigv = pool.tile([P, 1], f32)
    nc.vector.tensor_scalar(out=bigv, in0=valid, scalar1=-BIG,
                            scalar2=BIG, op0=Alu.mult, op1=Alu.add)
    nc.vector.tensor_tensor(out=mrow[:, 3:4], in0=bigv, in1=mrow[:, 2:3],
                            op=Alu.add)

    # ---- transpose each mrow column to a [1, P] row via PE ------------------
    # (separate [1,P] tiles so every later matmul operand sits at partition 0)
    onesrow = pool.tile([1, MP], f32)
    nc.vector.memset(onesrow, 1.0)
    mT = []
    for c in range(4):
        psTc = psum.tile([1, P], f32, tag=f"psT{c}")
        nc.tensor.matmul(out=psTc[:], lhsT=mrow[:, c:c + 1], rhs=idm[:],
                         is_transpose=True, start=True, stop=True)
        mTc = pool.tile([1, P], f32, tag=f"mT{c}")
        nc.vector.tensor_copy(out=mTc, in_=psTc)
        mT.append(mTc)
    meanT, neg2T, m2T, maskT = mT

    # ---- pairwise D'[i,j] = (mi-mj)^2 + BIG*(1-vj), per image ---------------
    psD = psum.tile([P, M], f32)
    for ni in range(NLOC):
        sl32 = slice(ni * MP, (ni + 1) * MP)
        sl30 = slice(ni * MP, ni * MP + M)
        Dni = psD[sl32, :]
        tp = (0, ni * MP)
        nc.tensor.matmul(out=Dni, lhsT=meanT[0:1, sl32], rhs=neg2T[0:1, sl30],
                         start=True, stop=False, tile_position=tp)   # -2*mi*mj
        nc.tensor.matmul(out=Dni, lhsT=m2T[0:1, sl32], rhs=onesrow[0:1, 0:M],
                         start=False, stop=False, tile_position=tp)  # + mi^2
        nc.tensor.matmul(out=Dni, lhsT=onesrow[0:1, :], rhs=maskT[0:1, sl30],
                         start=False, stop=True, tile_position=tp)   # + mj^2+BIG(1-vj)

    # ---- exp(-D') + row sums; diagonal contributes exactly v_i --------------
    pe = pool.tile([P, M], f32)
    rowsum = pool.tile([P, 1], f32)
    nc.scalar.activation(out=pe, in_=psD, func=mybir.ActivationFunctionType.Exp,
                         scale=-1.0, accum_out=rowsum)
    nc.vector.scalar_tensor_tensor(out=red[:, 2:3], in0=rowsum, scalar=valid,
                                   in1=valid, op0=Alu.mult, op1=Alu.subtract)

    # ---- per-image segment sums: [4,3] = seg.T @ red ------------------------
    psS = psum.tile([NLOC, 3], f32)
    nc.tensor.matmul(out=psS[:], lhsT=seg[:], rhs=red[:],
                     start=True, stop=True)
    s43 = pool.tile([NLOC, 3], f32)
    nc.vector.tensor_copy(out=s43, in_=psS)

    # ---- finals per image ---------------------------------------------------
    f42 = pool.tile([NLOC, 2], f32)
    nt = s43[:, 1:2]
    sant = pool.tile([NLOC, 1], f32)
    nc.vector.tensor_scalar(out=sant, in0=nt, scalar1=1.0, scalar2=None,
                            op0=Alu.max)
    rnt = pool.tile([NLOC, 1], f32)
    nc.vector.reciprocal(out=rnt, in_=sant)
    nc.vector.tensor_tensor(out=f42[:, 0:1], in0=s43[:, 0:1], in1=rnt,
                            op=Alu.mult)

    npr = pool.tile([NLOC, 1], f32)
    nc.vector.scalar_tensor_tensor(out=npr, in0=nt, scalar=-1.0, in1=nt,
                                   op0=Alu.add, op1=Alu.mult)   # (nt-1)*nt
    gate = pool.tile([NLOC, 1], f32)
    nc.vector.tensor_scalar(out=gate, in0=npr, scalar1=0.0, scalar2=None,
                            op0=Alu.is_gt)
    sanp = pool.tile([NLOC, 1], f32)
    nc.vector.tensor_scalar(out=sanp, in0=npr, scalar1=1.0, scalar2=None,
                            op0=Alu.max)
    rnp = pool.tile([NLOC, 1], f32)
    nc.vector.reciprocal(out=rnp, in_=sanp)
    t5 = pool.tile([NLOC, 1], f32)
    nc.vector.scalar_tensor_tensor(out=t5, in0=s43[:, 2:3], scalar=0.5,
                                   in1=rnp, op0=Alu.mult, op1=Alu.mult)
    nc.vector.tensor_tensor(out=f42[:, 1:2], in0=t5, in1=gate, op=Alu.mult)

    nc.sync.dma_start(out=out, in_=f42)

    if dbg:
        dbt = pool.tile([P, 24], f32)
        nc.vector.tensor_copy(out=dbt[:, 0:K], in_=g)
        nc.vector.tensor_copy(out=dbt[:, K:K + 3], in_=red)
        nc.vector.tensor_copy(out=dbt[:, K + 3:K + 7], in_=mrow)
        nc.sync.dma_start(out=dbg["f"][:], in_=dbt)
        dbi = pool.tile([P, K], i32)
        nc.vector.tensor_copy(out=dbi, in_=jt3[:, :, 0])
        nc.sync.dma_start(out=dbg["i"][:], in_=dbi)


_NC_CACHE = None


def _get_nc():
    global _NC_CACHE
    if _NC_CACHE is None:
        _NC_CACHE = build_nc()
    return _NC_CACHE


def make_in_maps(tags: np.ndarray, joints: np.ndarray):
    tags = np.ascontiguousarray(np.asarray(tags, dtype=np.float32))
    jt32 = np.asarray(joints).astype(np.int32)          # [N, M, K, 2]
    jt_pad = np.zeros((N, MP, K, 2), dtype=np.int32)    # rows 30,31 stay 0
    jt_pad[:, :M] = jt32
    # clip like jnp.take_along_axis does, then fold each image's flat-index
    # base into the loc channel (sharding step)
    np.clip(jt_pad[:, :, :, 0], 0, KHW - 1, out=jt_pad[:, :, :, 0])
    jt_pad[:, :, :, 0] += (np.arange(N, dtype=np.int32) % NLOC)[:, None, None] * KHW
    in_maps = []
    for c in range(NCORES):
        sl = slice(c * NLOC, (c + 1) * NLOC)
        in_maps.append({
            "tags": tags[sl].reshape(NLOC, KHW),
            "jt": np.ascontiguousarray(jt_pad[sl].reshape(P, K, 2)),
        })
    return in_maps


def kernel(tags: np.ndarray, joints: np.ndarray, _bench_results=None):
    nc = _get_nc()
    in_maps = make_in_maps(tags, joints)
    res = run_bass_kernel_spmd(nc, in_maps, core_ids=list(range(NCORES)))
    if _bench_results is not None:
        _bench_results.append(res)
    per_image = np.concatenate([r["out"] for r in res.results], axis=0)  # [32,2]
    pull_loss = np.float32(per_image[:, 0].mean(dtype=np.float64))
    push_loss = np.float32(per_image[:, 1].mean(dtype=np.float64))
    return pull_loss, push_loss



# revision 9
# speedup vs baseline: 1.7107x; 1.7107x over previous
"""Associative-embedding loss on 8 Trainium2 NeuronCores.

Data-parallel over batch N=32: each of the 8 cores handles 4 images.

Gather strategy: the per-image flat tag array (1114112 f32) is viewed as
17408 rows of 64 f32 (256B).  One SWDGE dma_gather per image fetches the
row containing each needed element (int16 row indices fit: 17408<32768).
Row indices and a {0,1} selection one-hot for the element-within-row are
precomputed on the host (pure address arithmetic).  Each image's
30 persons x 17 joints are spread over all 128 partitions (person
m <-> partitions 4m..4m+3, 5 slots each, 20 cells >= 17) so a single
gather delivers every partition's work.

On-chip pipeline per chunk (image): rows*onehot -> reduce(64) -> A[p,s];
gsv=A*vis, gsv2=gsv*A; a K=128 matmul against the constant indicator
IND32[p,m]=[p//4==m] reduces cells to per-person (sum v*g, sum v*g^2).
pull uses the variance identity; push replicates per-image (mean, valid)
rows to every partition with one K=128 matmul against a block-diagonal
constant, then exp(-(m_p-m_j)^2 - BIG*(1-v_j) - BIG*diag) on ACT with a
fused row-sum accumulator.  Per-image segment sums via one PE matmul.
All visibility-derived scalars (cnt, 1/cnt, valid, per-image pair counts)
are computed while the gathers are still in flight.
"""

import numpy as np
from contextlib import ExitStack

import concourse.bass as bass
import concourse.tile as tile
from concourse import library_config, mybir
from concourse.bass_utils import run_bass_kernel_spmd

# Problem constants (hardcoded per contract).
N, K, H, W, M = 32, 17, 256, 256, 30
NCORES = 8
NLOC = N // NCORES          # images per core
KHW = K * H * W             # 1114112 flat tag elements per image
MP = 32                     # padded persons per image
P = 128                     # SBUF partitions
BIG = 30.0                  # exp(-BIG) ~ 9e-14: kills masked columns

ROW = 64                    # gather row width (f32 elems, 256B)
NROWS = KHW // ROW          # 17408 rows per image (< int16 range)
SLOTS = 5                   # gather slots per partition per image
NI = SLOTS * P              # 640 idxs per per-image gather
CELLS = NLOC * SLOTS        # 20 scattered cells per partition

# blob column offsets
C_VIS = 0                   # [128, 20] raw vis values, scattered layout
C_VISP = C_VIS + CELLS      # [128, 17] raw vis values, person layout
C_PERS = C_VISP + K         # [128, 30] PERS30[p, j] = (p%32 == j)
C_BIGP = C_PERS + M         # [128, 30] BIG * (1 + PERS30)
C_IND = C_BIGP + M          # [128, 32] IND32[p, m] = (p//4 == m)
C_SEG = C_IND + MP          # [128, 4]  SEG[p, i] = (p//32==i and p%32<30)
C_IMG = C_SEG + NLOC        # [128, 128] IMGBLK[q, p] = (q//32 == p//32)
BLOB = C_IMG + P            # 261

f32 = mybir.dt.float32
i16 = mybir.dt.int16
Alu = mybir.AluOpType
Act = mybir.ActivationFunctionType
AxX = mybir.AxisListType.X


def build_nc() -> bass.Bass:
    nc = bass.Bass()
    tags_d = nc.declare_dram_parameter("tags", [NLOC, NROWS, ROW], f32,
                                       isOutput=False)
    idx_d = nc.declare_dram_parameter("idx", [P, NLOC * NI // 16], i16,
                                      isOutput=False)
    blob_d = nc.declare_dram_parameter("blob", [P, BLOB], f32, isOutput=False)
    oh_d = nc.declare_dram_parameter("oh", [P, CELLS * ROW], f32,
                                     isOutput=False)
    out_d = nc.declare_dram_parameter("out", [NLOC, 2], f32, isOutput=True)

    with tile.TileContext(nc) as tc:
        with ExitStack() as ctx:
            _body(ctx, tc, nc, tags_d, idx_d[:], blob_d[:], oh_d[:], out_d[:])
    _split_multi_waits(nc, max_waits=1)
    return nc


def _split_multi_waits(nc, max_waits=1):
    """Walrus codegen rejects instructions with too many sync-wait commands
    ("Too many sync wait commands", CoreV3GenImpl::setupSyncWait). Tile's
    kernel-tail drain waits on every live semaphore. Split the excess waits
    onto same-engine nops inserted immediately before the offending
    instruction — identical semantics, one wait per instruction."""
    import bass_rust
    fn = nc.m.functions[0]
    for bb in fn.blocks:
        changed = True
        while changed:
            changed = False
            for inst in list(bb.instructions):
                si = inst.sync_info
                if si is None or not si.on_wait or len(si.on_wait) <= max_waits:
                    continue
                waits = list(si.on_wait)
                keep, rest = waits[:max_waits], waits[max_waits:]
                nops = []
                for i in range(0, len(rest), max_waits):
                    nop_inst = nc.engines[inst.engine].nop().ins
                    nop_inst.sync_info = bass_rust.SyncInfo(
                        on_wait=rest[i:i + max_waits], on_update=[])
                    nops.append(nop_inst)
                inst.sync_info = bass_rust.SyncInfo(
                    on_wait=keep, on_update=list(si.on_update))
                for b2 in fn.blocks:
                    lst = b2.instructions
                    for i in range(len(lst) - 1, -1, -1):
                        if any(lst[i].name == n.name for n in nops):
                            del lst[i]
                idx = next(i for i, x in enumerate(bb.instructions)
                           if x.name == inst.name)
                for j, n in enumerate(nops):
                    bb.instructions.insert(idx + j, n)
                changed = True
                break


def _body(ctx, tc, nc, tags_d, idx, blob, oh, out):
    pool = ctx.enter_context(tc.tile_pool(name="main", bufs=1))
    psum = ctx.enter_context(tc.tile_pool(name="psum", bufs=1, space="PSUM"))

    # DMAGatherAnt lives in the 'mlp' Q7 library; load it before any gather.
    # The plain-Bass path never assembles the pseudo's instruction bytes
    # (only Bacc's codegen does), which walrus rejects as "ISA wrong length".
    # Assemble the NEURON_ISA_TPB_PSEUDO_LIBRARY_RELOAD_INDEX_STRUCT bytes
    # ourselves so walrus emits a runtime-interpreted PSEUDO_INST.
    import concourse.bass_isa as bass_isa
    reload_inst = nc.gpsimd.load_library(library_config.mlp)
    pseudo_op = nc.isa.get_enum("NEURON_ISA_TPB_PSEUDO_OPCODE")
    instr_bytes, _ = bass_isa.isa_struct(
        nc.isa, nc.isa.Opcode.NEURON_ISA_TPB_OPCODE_PSEUDO_INST,
        {"pseudo_opcode":
         pseudo_op.NEURON_ISA_TPB_PSEUDO_OPCODE_PSEUDO_LIBRARY_RELOAD_INDEX.value,
         "lib_index": library_config.mlp.index},
        struct_name="NEURON_ISA_TPB_PSEUDO_LIBRARY_RELOAD_INDEX_STRUCT")
    reload_inst.ins.instr = instr_bytes

    # ---- input DMAs ---------------------------------------------------------
    sb_idx = pool.tile([P, NLOC * NI // 16], i16)
    nc.sync.dma_start(out=sb_idx, in_=idx)
    sb_blob = pool.tile([P, BLOB], f32)
    nc.sync.dma_start(out=sb_blob, in_=blob)
    sb_oh = pool.tile([P, CELLS, ROW], f32)
    nc.sync.dma_start(out=sb_oh, in_=oh)

    vis_scat = sb_blob[:, C_VIS:C_VIS + CELLS]
    visp = sb_blob[:, C_VISP:C_VISP + K]
    pers30 = sb_blob[:, C_PERS:C_PERS + M]
    bigp = sb_blob[:, C_BIGP:C_BIGP + M]
    ind32 = sb_blob[:, C_IND:C_IND + MP]
    seg = sb_blob[:, C_SEG:C_SEG + NLOC]
    imgblk = sb_blob[:, C_IMG:C_IMG + P]

    # ---- per-image gathers (Pool SWDGE), issued ASAP ------------------------
    rows = []
    for c in range(NLOC):
        rows_c = pool.tile([P, SLOTS, ROW], f32, tag=f"rows{c}")
        nc.gpsimd.dma_gather(
            out_ap=rows_c[:],
            in_ap=tags_d[c],
            idxs_ap=sb_idx[:, c * (NI // 16):(c + 1) * (NI // 16)],
            num_idxs=NI,
            num_idxs_reg=NI,
            elem_size=ROW,
        )
        rows.append(rows_c)

    # ---- visibility-derived scalars (overlap the gathers) -------------------
    visf = pool.tile([P, CELLS], f32)      # scattered-layout 0/1 mask
    nc.vector.tensor_scalar(out=visf, in0=vis_scat, scalar1=0.0, scalar2=None,
                            op0=Alu.is_gt)
    visfp = pool.tile([P, K], f32)         # person-layout 0/1 mask
    nc.vector.tensor_scalar(out=visfp, in0=visp, scalar1=0.0, scalar2=None,
                            op0=Alu.is_gt)
    cnt = pool.tile([P, 1], f32)
    nc.vector.reduce_sum(out=cnt, in_=visfp, axis=AxX)
    valid = pool.tile([P, 1], f32)
    nc.vector.tensor_scalar(out=valid, in0=cnt, scalar1=1.0, scalar2=None,
                            op0=Alu.min)
    safecnt = pool.tile([P, 1], f32)
    nc.vector.tensor_scalar(out=safecnt, in0=cnt, scalar1=1.0, scalar2=None,
                            op0=Alu.max)
    rc = pool.tile([P, 1], f32)
    nc.vector.reciprocal(out=rc, in_=safecnt)

    rhs60 = pool.tile([P, 2 * M], f32)     # cols 0:30 mean*PERS30, 30:60 valid*PERS30
    nc.vector.tensor_scalar(out=rhs60[:, M:2 * M], in0=pers30, scalar1=valid,
                            scalar2=None, op0=Alu.mult)

    # per-image tag counts + push denominators (all early, off critical path)
    psNT = psum.tile([NLOC, 1], f32, tag="psNT")
    nc.tensor.matmul(out=psNT[:], lhsT=seg, rhs=valid, start=True, stop=True)
    nt = pool.tile([NLOC, 1], f32)
    nc.vector.tensor_copy(out=nt, in_=psNT)
    sant = pool.tile([NLOC, 1], f32)
    nc.vector.tensor_scalar(out=sant, in0=nt, scalar1=1.0, scalar2=None,
                            op0=Alu.max)
    rnt = pool.tile([NLOC, 1], f32)
    nc.vector.reciprocal(out=rnt, in_=sant)
    npr = pool.tile([NLOC, 1], f32)
    nc.vector.scalar_tensor_tensor(out=npr, in0=nt, scalar=-1.0, in1=nt,
                                   op0=Alu.add, op1=Alu.mult)  # (nt-1)*nt
    gate = pool.tile([NLOC, 1], f32)
    nc.vector.tensor_scalar(out=gate, in0=npr, scalar1=0.0, scalar2=None,
                            op0=Alu.is_gt)
    sanp = pool.tile([NLOC, 1], f32)
    nc.vector.tensor_scalar(out=sanp, in0=npr, scalar1=1.0, scalar2=None,
                            op0=Alu.max)
    rnp = pool.tile([NLOC, 1], f32)
    nc.vector.reciprocal(out=rnp, in_=sanp)
    grnp = pool.tile([NLOC, 1], f32)
    nc.vector.scalar_tensor_tensor(out=grnp, in0=gate, scalar=0.5, in1=rnp,
                                   op0=Alu.mult, op1=Alu.mult)  # 0.5*gate*rnp

    # ---- per-chunk selection + cell stats + person-stats matmul -------------
    A = pool.tile([P, CELLS], f32)
    GV = pool.tile([P, 2, CELLS], f32)
    psS = psum.tile([P, 2, SLOTS], f32, tag="psS")

    for c in range(NLOC):
        cs = slice(c * SLOTS, (c + 1) * SLOTS)
        sel_c = pool.tile([P, SLOTS, ROW], f32, tag=f"sel{c}")
        nc.vector.tensor_tensor(out=sel_c, in0=rows[c][:],
                                in1=sb_oh[:, cs, :], op=Alu.mult)
        nc.vector.reduce_sum(out=A[:, cs], in_=sel_c[:], axis=AxX)
        nc.vector.tensor_tensor(out=GV[:, 0, cs], in0=A[:, cs],
                                in1=visf[:, cs], op=Alu.mult)
        nc.vector.tensor_tensor(out=GV[:, 1, cs], in0=GV[:, 0, cs],
                                in1=A[:, cs], op=Alu.mult)
        nc.tensor.matmul(out=psS[c * MP:(c + 1) * MP, :, :], lhsT=ind32,
                         rhs=GV[:, :, cs], start=True, stop=True,
                         tile_position=(0, c * MP))

    X = pool.tile([P, 2], f32)             # col0 = sum v*g, col1 = sum v*g^2
    nc.vector.reduce_sum(out=X, in_=psS[:], axis=AxX)

    # ---- per-person mean / pull ---------------------------------------------
    mean = pool.tile([P, 1], f32)
    nc.vector.tensor_tensor(out=mean, in0=X[:, 0:1], in1=rc, op=Alu.mult)
    nc.vector.tensor_scalar(out=rhs60[:, 0:M], in0=pers30, scalar1=mean,
                            scalar2=None, op0=Alu.mult)
    mean2 = pool.tile([P, 1], f32)
    nc.vector.tensor_tensor(out=mean2, in0=mean, in1=mean, op=Alu.mult)
    p1 = pool.tile([P, 1], f32)
    nc.vector.scalar_tensor_tensor(out=p1, in0=X[:, 1:2], scalar=rc,
                                   in1=mean2, op0=Alu.mult, op1=Alu.subtract)
    red = pool.tile([P, 2], f32)
    nc.vector.tensor_tensor(out=red[:, 0:1], in0=p1, in1=valid, op=Alu.mult)

    # ---- push: replicate per-image (mean, valid) rows, pairwise exp ---------
    psRep = psum.tile([P, 2 * M], f32, tag="psRep")
    nc.tensor.matmul(out=psRep[:], lhsT=imgblk, rhs=rhs60[:],
                     start=True, stop=True)
    t = pool.tile([P, M], f32)
    nc.vector.tensor_scalar(out=t, in0=psRep[:, 0:M], scalar1=mean,
                            scalar2=None, op0=Alu.subtract)
    bigrow = pool.tile([P, M], f32)
    nc.vector.scalar_tensor_tensor(out=bigrow, in0=psRep[:, M:2 * M],
                                   scalar=-BIG, in1=bigp, op0=Alu.mult,
                                   op1=Alu.add)   # BIG*(1-vj) + BIG*diag
    t2 = pool.tile([P, M], f32)
    nc.vector.tensor_tensor(out=t2, in0=t, in1=t, op=Alu.mult)
    t2b = pool.tile([P, M], f32)
    nc.vector.tensor_tensor(out=t2b, in0=t2, in1=bigrow, op=Alu.add)
    pe = pool.tile([P, M], f32)
    rowsum = pool.tile([P, 1], f32)
    nc.scalar.activation(out=pe, in_=t2b, func=Act.Exp, scale=-1.0,
                         accum_out=rowsum)
    nc.vector.tensor_tensor(out=red[:, 1:2], in0=rowsum, in1=valid,
                            op=Alu.mult)

    # ---- per-image segment sums + finals ------------------------------------
    psF = psum.tile([NLOC, 2], f32, tag="psF")
    nc.tensor.matmul(out=psF[:], lhsT=seg, rhs=red[:], start=True, stop=True)
    f42 = pool.tile([NLOC, 2], f32)
    nc.vector.tensor_tensor(out=f42[:, 0:1], in0=psF[:, 0:1], in1=rnt,
                            op=Alu.mult)
    nc.vector.tensor_tensor(out=f42[:, 1:2], in0=psF[:, 1:2], in1=grnp,
                            op=Alu.mult)
    nc.sync.dma_start(out=out, in_=f42)


# ---------------------------------------------------------------------------
# host side
# ---------------------------------------------------------------------------

def _build_consts():
    p = np.arange(P)
    j30 = np.arange(M)
    pers30 = ((p[:, None] % MP) == j30[None, :]).astype(np.float32)
    bigp = BIG * (1.0 + pers30)
    ind32 = ((p[:, None] // 4) == np.arange(MP)[None, :]).astype(np.float32)
    seg = (((p[:, None] // MP) == np.arange(NLOC)[None, :])
           & ((p[:, None] % MP) < M)).astype(np.float32)
    imgblk = ((p[:, None] // MP) == (p[None, :] // MP)).astype(np.float32)
    return np.concatenate([pers30, bigp, ind32, seg, imgblk],
                          axis=1).astype(np.float32)  # [128, 224]


_CONSTS = _build_consts()

# cell mapping: joint k of person m -> partition 4m + k//5, slot k%5
_MM, _KK = np.meshgrid(np.arange(M), np.arange(K), indexing="ij")
_CELL_P = (4 * _MM + _KK // SLOTS).ravel()          # [510]
_CELL_S = (_KK % SLOTS).ravel()                     # [510]
_IDX_I = (_CELL_S * P + _CELL_P)                    # gather idx position i


def make_in_maps(tags: np.ndarray, joints: np.ndarray):
    tags = np.ascontiguousarray(np.asarray(tags, dtype=np.float32))
    jt = np.asarray(joints)
    loc = np.clip(jt[..., 0], 0, KHW - 1).astype(np.int64)   # [N, M, K]
    visraw = jt[..., 1].astype(np.float32)                   # [N, M, K]
    row = (loc // ROW).astype(np.int16)                      # [N, M, K]
    sub = (loc % ROW).astype(np.int64)                       # [N, M, K]

    # gather idx arrays [N, 640] -> SBUF wrap [N, 128, 40]
    idx_all = np.zeros((N, NI), dtype=np.int16)
    idx_all[:, _IDX_I] = row.reshape(N, M * K)
    idx_sb = np.tile(idx_all.reshape(N, NI // 16, 16).transpose(0, 2, 1),
                     (1, 8, 1))                              # [N, 128, 40]

    # selection one-hot [N, 128, SLOTS*ROW]
    onehot = np.zeros((N, P, SLOTS * ROW), dtype=np.float32)
    cellflat = (_CELL_S * ROW)[None, :] + sub.reshape(N, M * K)  # [N, 510]
    onehot[np.arange(N)[:, None], _CELL_P[None, :], cellflat] = 1.0

    # scattered vis [N, 128, SLOTS]
    vis_scat = np.zeros((N, P, SLOTS), dtype=np.float32)
    vis_scat[np.arange(N)[:, None], _CELL_P[None, :],
             _CELL_S[None, :]] = visraw.reshape(N, M * K)

    # person-layout vis [N, 32, K]
    visp = np.zeros((N, MP, K), dtype=np.float32)
    visp[:, :M, :] = visraw

    in_maps = []
    for c in range(NCORES):
        sl = slice(c * NLOC, (c + 1) * NLOC)
        blob = np.concatenate([
            vis_scat[sl].transpose(1, 0, 2).reshape(P, CELLS),
            visp[sl].reshape(P, K),
            _CONSTS,
        ], axis=1)
        in_maps.append({
            "tags": tags[sl].reshape(NLOC, NROWS, ROW),
            "idx": np.ascontiguousarray(
                np.concatenate(idx_sb[sl], axis=1)),     # [128, 160]
            "blob": np.ascontiguousarray(blob),
            "oh": np.ascontiguousarray(
                onehot[sl].transpose(1, 0, 2).reshape(P, CELLS * ROW)),
        })
    return in_maps


_NC_CACHE = None


def _get_nc():
    global _NC_CACHE
    if _NC_CACHE is None:
        _NC_CACHE = build_nc()
    return _NC_CACHE


def kernel(tags: np.ndarray, joints: np.ndarray, _bench_results=None):
    nc = _get_nc()
    in_maps = make_in_maps(tags, joints)
    res = run_bass_kernel_spmd(nc, in_maps, core_ids=list(range(NCORES)))
    if _bench_results is not None:
        _bench_results.append(res)
    per_image = np.concatenate([r["out"] for r in res.results], axis=0)
    pull_loss = np.float32(per_image[:, 0].mean(dtype=np.float64))
    push_loss = np.float32(per_image[:, 1].mean(dtype=np.float64))
    return pull_loss, push_loss


# revision 13
# speedup vs baseline: 1.8146x; 1.0608x over previous
"""Associative-embedding loss on 8 Trainium2 NeuronCores.

Data-parallel over batch N=32: each of the 8 cores handles 4 images.

Gather strategy: the per-image flat tag array (1114112 f32) is viewed as
17408 rows of 64 f32 (256B).  One SWDGE dma_gather per image fetches the
row containing each needed element (int16 row indices fit: 17408<32768).
Row indices and a {0,1} selection one-hot for the element-within-row are
precomputed on the host (pure address arithmetic).  Each image's
30 persons x 17 joints are spread over all 128 partitions (person
m <-> partitions 4m..4m+3, 5 slots each, 20 cells >= 17) so a single
gather delivers every partition's work.

On-chip pipeline per chunk (image): rows*onehot -> reduce(64) -> A[p,s];
gsv=A*vis, gsv2=gsv*A; a K=128 matmul against the constant indicator
IND32[p,m]=[p//4==m] reduces cells to per-person (sum v*g, sum v*g^2).
pull uses the variance identity; push replicates per-image (mean, valid)
rows to every partition with one K=128 matmul against a block-diagonal
constant, then exp(-(m_p-m_j)^2 - BIG*(1-v_j) - BIG*diag) on ACT with a
fused row-sum accumulator.  Per-image segment sums via one PE matmul.
All visibility-derived scalars (cnt, 1/cnt, valid, per-image pair counts)
are computed while the gathers are still in flight.
"""

import numpy as np
from contextlib import ExitStack

import concourse.bass as bass
import concourse.tile as tile
from concourse import library_config, mybir
from concourse.bass_utils import run_bass_kernel_spmd

# Problem constants (hardcoded per contract).
N, K, H, W, M = 32, 17, 256, 256, 30
NCORES = 8
NLOC = N // NCORES          # images per core
KHW = K * H * W             # 1114112 flat tag elements per image
MP = 32                     # padded persons per image
P = 128                     # SBUF partitions
BIG = 30.0                  # exp(-BIG) ~ 9e-14: kills masked columns

ROW = 64                    # gather row width (f32 elems, 256B)
NROWS = KHW // ROW          # 17408 rows per image (< int16 range)
SLOTS = 5                   # gather slots per partition per image
NI = SLOTS * P              # 640 idxs per per-image gather
CELLS = NLOC * SLOTS        # 20 scattered cells per partition

# blob column offsets
C_VIS = 0                   # [128, 20] raw vis values, scattered layout
C_VISP = C_VIS + CELLS      # [128, 17] raw vis values, person layout
C_PERS = C_VISP + K         # [128, 30] PERS30[p, j] = (p%32 == j)
C_IND = C_PERS + M          # [128, 32] IND32[p, m] = (p//4 == m)
C_SEG = C_IND + MP          # [128, 4]  SEG[p, i] = (p//32==i and p%32<30)
C_IMG = C_SEG + NLOC        # [128, 128] IMGBLK[q, p] = (q//32 == p//32)
BLOB = C_IMG + P            # 231

# push math via Derivative_Erf(x) = (2/sqrt(pi)) * exp(-x^2):
#   arg[p,j] = mean_j + BIGK*(1-valid_j) - mean_p
# valid j != p: exp(-(mean_j-mean_p)^2); invalid j: exp(-(~BIGK)^2) ~ 0;
# diagonal (valid p): derf(0) = 2/sqrt(pi), removed per image via
# psF1 - (2/sqrt(pi))*nt before scaling by sqrt(pi)/2.
SQPI_4 = 0.44311346272637900682   # sqrt(pi)/4

f32 = mybir.dt.float32
i16 = mybir.dt.int16
Alu = mybir.AluOpType
Act = mybir.ActivationFunctionType
AxX = mybir.AxisListType.X


def build_nc() -> bass.Bass:
    nc = bass.Bass()
    tags_d = nc.declare_dram_parameter("tags", [NLOC, NROWS, ROW], f32,
                                       isOutput=False)
    idx_d = nc.declare_dram_parameter("idx", [P, NLOC * NI // 16], i16,
                                      isOutput=False)
    blob_d = nc.declare_dram_parameter("blob", [P, BLOB], f32, isOutput=False)
    oh_d = nc.declare_dram_parameter("oh", [P, CELLS * ROW], f32,
                                     isOutput=False)
    out_d = nc.declare_dram_parameter("out", [NLOC, 2], f32, isOutput=True)

    with tile.TileContext(nc) as tc:
        with ExitStack() as ctx:
            _body(ctx, tc, nc, tags_d, idx_d[:], blob_d[:], oh_d[:], out_d[:])
    _split_multi_waits(nc, max_waits=1)
    return nc


def _split_multi_waits(nc, max_waits=1):
    """Walrus codegen rejects instructions with too many sync-wait commands
    ("Too many sync wait commands", CoreV3GenImpl::setupSyncWait). Tile's
    kernel-tail drain waits on every live semaphore. Split the excess waits
    onto same-engine nops inserted immediately before the offending
    instruction — identical semantics, one wait per instruction."""
    import bass_rust
    fn = nc.m.functions[0]
    for bb in fn.blocks:
        changed = True
        while changed:
            changed = False
            for inst in list(bb.instructions):
                si = inst.sync_info
                if si is None or not si.on_wait or len(si.on_wait) <= max_waits:
                    continue
                waits = list(si.on_wait)
                keep, rest = waits[:max_waits], waits[max_waits:]
                nops = []
                for i in range(0, len(rest), max_waits):
                    nop_inst = nc.engines[inst.engine].nop().ins
                    nop_inst.sync_info = bass_rust.SyncInfo(
                        on_wait=rest[i:i + max_waits], on_update=[])
                    nops.append(nop_inst)
                inst.sync_info = bass_rust.SyncInfo(
                    on_wait=keep, on_update=list(si.on_update))
                for b2 in fn.blocks:
                    lst = b2.instructions
                    for i in range(len(lst) - 1, -1, -1):
                        if any(lst[i].name == n.name for n in nops):
                            del lst[i]
                idx = next(i for i, x in enumerate(bb.instructions)
                           if x.name == inst.name)
                for j, n in enumerate(nops):
                    bb.instructions.insert(idx + j, n)
                changed = True
                break


def _body(ctx, tc, nc, tags_d, idx, blob, oh, out):
    pool = ctx.enter_context(tc.tile_pool(name="main", bufs=1))
    psum = ctx.enter_context(tc.tile_pool(name="psum", bufs=1, space="PSUM"))

    # DMAGatherAnt lives in the 'mlp' Q7 library; load it before any gather.
    # The plain-Bass path never assembles the pseudo's instruction bytes
    # (only Bacc's codegen does), which walrus rejects as "ISA wrong length".
    # Assemble the NEURON_ISA_TPB_PSEUDO_LIBRARY_RELOAD_INDEX_STRUCT bytes
    # ourselves so walrus emits a runtime-interpreted PSEUDO_INST.
    import concourse.bass_isa as bass_isa
    reload_inst = nc.gpsimd.load_library(library_config.mlp)
    pseudo_op = nc.isa.get_enum("NEURON_ISA_TPB_PSEUDO_OPCODE")
    instr_bytes, _ = bass_isa.isa_struct(
        nc.isa, nc.isa.Opcode.NEURON_ISA_TPB_OPCODE_PSEUDO_INST,
        {"pseudo_opcode":
         pseudo_op.NEURON_ISA_TPB_PSEUDO_OPCODE_PSEUDO_LIBRARY_RELOAD_INDEX.value,
         "lib_index": library_config.mlp.index},
        struct_name="NEURON_ISA_TPB_PSEUDO_LIBRARY_RELOAD_INDEX_STRUCT")
    reload_inst.ins.instr = instr_bytes

    # ---- input DMAs ---------------------------------------------------------
    sb_idx = pool.tile([P, NLOC * NI // 16], i16)
    nc.sync.dma_start(out=sb_idx, in_=idx)
    sb_blob = pool.tile([P, BLOB], f32)
    nc.sync.dma_start(out=sb_blob, in_=blob)
    sb_oh = pool.tile([P, CELLS, ROW], f32)
    nc.sync.dma_start(out=sb_oh, in_=oh)

    vis_scat = sb_blob[:, C_VIS:C_VIS + CELLS]
    visp = sb_blob[:, C_VISP:C_VISP + K]
    pers30 = sb_blob[:, C_PERS:C_PERS + M]
    ind32 = sb_blob[:, C_IND:C_IND + MP]
    seg = sb_blob[:, C_SEG:C_SEG + NLOC]
    imgblk = sb_blob[:, C_IMG:C_IMG + P]

    # ---- per-image gathers (Pool SWDGE), issued ASAP ------------------------
    rows = []
    for c in range(NLOC):
        rows_c = pool.tile([P, SLOTS, ROW], f32, tag=f"rows{c}")
        nc.gpsimd.dma_gather(
            out_ap=rows_c[:],
            in_ap=tags_d[c],
            idxs_ap=sb_idx[:, c * (NI // 16):(c + 1) * (NI // 16)],
            num_idxs=NI,
            num_idxs_reg=NI,
            elem_size=ROW,
        )
        rows.append(rows_c)

    # ---- visibility-derived scalars (overlap the gathers) -------------------
    visf = pool.tile([P, CELLS], f32)      # scattered-layout 0/1 mask
    nc.vector.tensor_scalar(out=visf, in0=vis_scat, scalar1=0.0, scalar2=None,
                            op0=Alu.is_gt)
    visfp = pool.tile([P, K], f32)         # person-layout 0/1 mask
    nc.vector.tensor_scalar(out=visfp, in0=visp, scalar1=0.0, scalar2=None,
                            op0=Alu.is_gt)
    cnt = pool.tile([P, 1], f32)
    nc.vector.reduce_sum(out=cnt, in_=visfp, axis=AxX)
    valid = pool.tile([P, 1], f32)
    nc.vector.tensor_scalar(out=valid, in0=cnt, scalar1=1.0, scalar2=None,
                            op0=Alu.min)
    safecnt = pool.tile([P, 1], f32)
    nc.vector.tensor_scalar(out=safecnt, in0=cnt, scalar1=1.0, scalar2=None,
                            op0=Alu.max)
    rc = pool.tile([P, 1], f32)
    nc.vector.reciprocal(out=rc, in_=safecnt)
    sm = pool.tile([P, 1], f32)            # BIGK*(1-valid)
    nc.vector.tensor_scalar(out=sm, in0=valid, scalar1=-BIG, scalar2=BIG,
                            op0=Alu.mult, op1=Alu.add)
    segv = pool.tile([P, NLOC], f32)       # SEG gated by person validity
    nc.vector.tensor_scalar(out=segv, in0=seg, scalar1=valid, scalar2=None,
                            op0=Alu.mult)

    # per-image tag counts + push denominators (all early, off critical path)
    psNT = psum.tile([NLOC, 1], f32, tag="psNT")
    nc.tensor.matmul(out=psNT[:], lhsT=seg, rhs=valid, start=True, stop=True)
    nt = pool.tile([NLOC, 1], f32)
    nc.vector.tensor_copy(out=nt, in_=psNT)
    sant = pool.tile([NLOC, 1], f32)
    nc.vector.tensor_scalar(out=sant, in0=nt, scalar1=1.0, scalar2=None,
                            op0=Alu.max)
    rnt = pool.tile([NLOC, 1], f32)
    nc.vector.reciprocal(out=rnt, in_=sant)
    npr = pool.tile([NLOC, 1], f32)
    nc.vector.scalar_tensor_tensor(out=npr, in0=nt, scalar=-1.0, in1=nt,
                                   op0=Alu.add, op1=Alu.mult)  # (nt-1)*nt
    gate = pool.tile([NLOC, 1], f32)
    nc.vector.tensor_scalar(out=gate, in0=npr, scalar1=0.0, scalar2=None,
                            op0=Alu.is_gt)
    sanp = pool.tile([NLOC, 1], f32)
    nc.vector.tensor_scalar(out=sanp, in0=npr, scalar1=1.0, scalar2=None,
                            op0=Alu.max)
    rnp = pool.tile([NLOC, 1], f32)
    nc.vector.reciprocal(out=rnp, in_=sanp)
    ga = pool.tile([NLOC, 1], f32)         # sqrt(pi)/4 * gate * rnp
    nc.vector.scalar_tensor_tensor(out=ga, in0=gate, scalar=SQPI_4, in1=rnp,
                                   op0=Alu.mult, op1=Alu.mult)
    gb0 = pool.tile([NLOC, 1], f32)
    nc.vector.scalar_tensor_tensor(out=gb0, in0=rnp, scalar=0.5, in1=nt,
                                   op0=Alu.mult, op1=Alu.mult)
    gb = pool.tile([NLOC, 1], f32)         # 0.5 * nt * gate * rnp
    nc.vector.tensor_tensor(out=gb, in0=gb0, in1=gate, op=Alu.mult)

    # ---- per-chunk selection + cell stats + person-stats matmul -------------
    A = pool.tile([P, CELLS], f32)
    GV = pool.tile([P, 2, CELLS], f32)
    GVR = pool.tile([P, NLOC, 2], f32)     # per-chunk slot-reduced (S1,S2)
    psS = psum.tile([P, 2], f32, tag="psS")

    for c in range(NLOC):
        cs = slice(c * SLOTS, (c + 1) * SLOTS)
        sel_c = pool.tile([P, SLOTS, ROW], f32, tag=f"sel{c}")
        nc.vector.tensor_tensor(out=sel_c, in0=rows[c][:],
                                in1=sb_oh[:, cs, :], op=Alu.mult)
        nc.vector.reduce_sum(out=A[:, cs], in_=sel_c[:], axis=AxX)
        nc.vector.tensor_tensor(out=GV[:, 0, cs], in0=A[:, cs],
                                in1=visf[:, cs], op=Alu.mult)
        nc.vector.tensor_tensor(out=GV[:, 1, cs], in0=GV[:, 0, cs],
                                in1=A[:, cs], op=Alu.mult)
        nc.vector.reduce_sum(out=GVR[:, c, :], in_=GV[:, :, cs], axis=AxX)
        nc.tensor.matmul(out=psS[c * MP:(c + 1) * MP, :], lhsT=ind32,
                         rhs=GVR[:, c, :], start=True, stop=True,
                         tile_position=(0, c * MP))

    # ---- per-person mean / push arg / pull ----------------------------------
    smm = pool.tile([P, 1], f32)           # mean + BIGK*(1-valid)
    nc.vector.scalar_tensor_tensor(out=smm, in0=psS[:, 0:1], scalar=rc,
                                   in1=sm, op0=Alu.mult, op1=Alu.add)
    mrhs = pool.tile([P, M], f32)
    nc.vector.tensor_scalar(out=mrhs, in0=pers30, scalar1=smm, scalar2=None,
                            op0=Alu.mult)
    psRep = psum.tile([P, M], f32, tag="psRep")
    nc.tensor.matmul(out=psRep[:], lhsT=imgblk, rhs=mrhs[:],
                     start=True, stop=True)
    negmean = pool.tile([P, 1], f32)
    nc.vector.tensor_scalar(out=negmean, in0=psS[:, 0:1], scalar1=rc,
                            scalar2=-1.0, op0=Alu.mult, op1=Alu.mult)
    mean2 = pool.tile([P, 1], f32)
    nc.vector.tensor_tensor(out=mean2, in0=negmean, in1=negmean, op=Alu.mult)
    p1 = pool.tile([P, 1], f32)
    nc.vector.scalar_tensor_tensor(out=p1, in0=psS[:, 1:2], scalar=rc,
                                   in1=mean2, op0=Alu.mult, op1=Alu.subtract)
    pullred = pool.tile([P, 1], f32)
    nc.vector.tensor_tensor(out=pullred, in0=p1, in1=valid, op=Alu.mult)

    # derf(arg) = (2/sqrt(pi)) exp(-arg^2); row sum in the ACT accumulator
    pe = pool.tile([P, M], f32)
    rowsumv = pool.tile([P, 1], f32)
    nc.scalar.activation(out=pe, in_=psRep[:], func=Act.Derivative_Erf,
                         bias=negmean, accum_out=rowsumv)

    # ---- per-image segment sums + finals ------------------------------------
    psF = psum.tile([NLOC, 2], f32, tag="psF")
    nc.tensor.matmul(out=psF[:, 0:1], lhsT=seg, rhs=pullred[:],
                     start=True, stop=True)
    nc.tensor.matmul(out=psF[:, 1:2], lhsT=segv, rhs=rowsumv[:],
                     start=True, stop=True)
    f42 = pool.tile([NLOC, 2], f32)
    nc.vector.tensor_scalar(out=f42[:, 0:1], in0=psF[:, 0:1], scalar1=rnt,
                            scalar2=None, op0=Alu.mult)
    nc.vector.scalar_tensor_tensor(out=f42[:, 1:2], in0=psF[:, 1:2],
                                   scalar=ga, in1=gb, op0=Alu.mult,
                                   op1=Alu.subtract)
    nc.sync.dma_start(out=out, in_=f42)


# ---------------------------------------------------------------------------
# host side
# ---------------------------------------------------------------------------

def _build_consts():
    p = np.arange(P)
    j30 = np.arange(M)
    pers30 = ((p[:, None] % MP) == j30[None, :]).astype(np.float32)
    ind32 = ((p[:, None] // 4) == np.arange(MP)[None, :]).astype(np.float32)
    seg = (((p[:, None] // MP) == np.arange(NLOC)[None, :])
           & ((p[:, None] % MP) < M)).astype(np.float32)
    imgblk = ((p[:, None] // MP) == (p[None, :] // MP)).astype(np.float32)
    return np.concatenate([pers30, ind32, seg, imgblk],
                          axis=1).astype(np.float32)  # [128, 194]


_CONSTS = _build_consts()

# cell mapping: joint k of person m -> partition 4m + k//5, slot k%5
_MM, _KK = np.meshgrid(np.arange(M), np.arange(K), indexing="ij")
_CELL_P = (4 * _MM + _KK // SLOTS).ravel()          # [510]
_CELL_S = (_KK % SLOTS).ravel()                     # [510]
_IDX_I = (_CELL_S * P + _CELL_P)                    # gather idx position i


def make_in_maps(tags: np.ndarray, joints: np.ndarray):
    tags = np.ascontiguousarray(np.asarray(tags, dtype=np.float32))
    jt = np.asarray(joints)
    loc = np.clip(jt[..., 0], 0, KHW - 1).astype(np.int64)   # [N, M, K]
    visraw = jt[..., 1].astype(np.float32)                   # [N, M, K]
    row = (loc // ROW).astype(np.int16)                      # [N, M, K]
    sub = (loc % ROW).astype(np.int64)                       # [N, M, K]

    # gather idx arrays [N, 640] -> SBUF wrap [N, 128, 40]
    idx_all = np.zeros((N, NI), dtype=np.int16)
    idx_all[:, _IDX_I] = row.reshape(N, M * K)
    idx_sb = np.tile(idx_all.reshape(N, NI // 16, 16).transpose(0, 2, 1),
                     (1, 8, 1))                              # [N, 128, 40]

    # selection one-hot [N, 128, SLOTS*ROW]
    onehot = np.zeros((N, P, SLOTS * ROW), dtype=np.float32)
    cellflat = (_CELL_S * ROW)[None, :] + sub.reshape(N, M * K)  # [N, 510]
    onehot[np.arange(N)[:, None], _CELL_P[None, :], cellflat] = 1.0

    # scattered vis [N, 128, SLOTS]
    vis_scat = np.zeros((N, P, SLOTS), dtype=np.float32)
    vis_scat[np.arange(N)[:, None], _CELL_P[None, :],
             _CELL_S[None, :]] = visraw.reshape(N, M * K)

    # person-layout vis [N, 32, K]
    visp = np.zeros((N, MP, K), dtype=np.float32)
    visp[:, :M, :] = visraw

    in_maps = []
    for c in range(NCORES):
        sl = slice(c * NLOC, (c + 1) * NLOC)
        blob = np.concatenate([
            vis_scat[sl].transpose(1, 0, 2).reshape(P, CELLS),
            visp[sl].reshape(P, K),
            _CONSTS,
        ], axis=1)
        in_maps.append({
            "tags": tags[sl].reshape(NLOC, NROWS, ROW),
            "idx": np.ascontiguousarray(
                np.concatenate(idx_sb[sl], axis=1)),     # [128, 160]
            "blob": np.ascontiguousarray(blob),
            "oh": np.ascontiguousarray(
                onehot[sl].transpose(1, 0, 2).reshape(P, CELLS * ROW)),
        })
    return in_maps


_NC_CACHE = None


def _get_nc():
    global _NC_CACHE
    if _NC_CACHE is None:
        _NC_CACHE = build_nc()
    return _NC_CACHE


def kernel(tags: np.ndarray, joints: np.ndarray, _bench_results=None):
    nc = _get_nc()
    in_maps = make_in_maps(tags, joints)
    res = run_bass_kernel_spmd(nc, in_maps, core_ids=list(range(NCORES)))
    if _bench_results is not None:
        _bench_results.append(res)
    per_image = np.concatenate([r["out"] for r in res.results], axis=0)
    pull_loss = np.float32(per_image[:, 0].mean(dtype=np.float64))
    push_loss = np.float32(per_image[:, 1].mean(dtype=np.float64))
    return pull_loss, push_loss


# revision 19
# speedup vs baseline: 1.8338x; 1.0106x over previous
"""Associative-embedding loss on 8 Trainium2 NeuronCores.

Data-parallel over batch N=32: each of the 8 cores handles 4 images.

Gather strategy: the per-image flat tag array (1114112 f32) is viewed as
17408 rows of 64 f32 (256B).  One SWDGE dma_gather per image fetches the
row containing each needed element (int16 row indices fit: 17408<32768).
Row indices and a {0,1} selection one-hot for the element-within-row are
precomputed on the host (pure address arithmetic).  Each image's
30 persons x 17 joints are spread over all 128 partitions (person
m <-> partitions 4m..4m+3, 5 slots each, 20 cells >= 17) so a single
gather delivers every partition's work.

On-chip pipeline per chunk (image): rows*onehot -> reduce(64) -> A[p,s];
gsv=A*vis, gsv2=gsv*A; a K=128 matmul against the constant indicator
IND32[p,m]=[p//4==m] reduces cells to per-person (sum v*g, sum v*g^2).
pull uses the variance identity; push replicates per-image (mean, valid)
rows to every partition with one K=128 matmul against a block-diagonal
constant, then exp(-(m_p-m_j)^2 - BIG*(1-v_j) - BIG*diag) on ACT with a
fused row-sum accumulator.  Per-image segment sums via one PE matmul.
All visibility-derived scalars (cnt, 1/cnt, valid, per-image pair counts)
are computed while the gathers are still in flight.
"""

import numpy as np
import ml_dtypes
from contextlib import ExitStack

import concourse.bass as bass
import concourse.tile as tile
from concourse import library_config, mybir
from concourse.bass_utils import run_bass_kernel_spmd

# Problem constants (hardcoded per contract).
N, K, H, W, M = 32, 17, 256, 256, 30
NCORES = 8
NLOC = N // NCORES          # images per core
KHW = K * H * W             # 1114112 flat tag elements per image
MP = 32                     # padded persons per image
P = 128                     # SBUF partitions
BIG = 30.0                  # exp(-BIG) ~ 9e-14: kills masked columns

ROW = 64                    # gather row width (f32 elems, 256B)
NROWS = KHW // ROW          # 17408 rows per image (< int16 range)
SLOTS = 5                   # gather slots per partition per image
NI = SLOTS * P              # 640 idxs per per-image gather
CELLS = NLOC * SLOTS        # 20 scattered cells per partition

# blob column offsets
C_VIS = 0                   # [128, 20] raw vis values, scattered layout
C_VISP = C_VIS + CELLS      # [128, 17] raw vis values, person layout
C_PERS = C_VISP + K         # [128, 30] PERS30[p, j] = (p%32 == j)
C_IND = C_PERS + M          # [128, 32] IND32[p, m] = (p//4 == m)
C_SEG = C_IND + MP          # [128, 4]  SEG[p, i] = (p//32==i and p%32<30)
C_IMG = C_SEG + NLOC        # [128, 128] IMGBLK[q, p] = (q//32 == p//32)
BLOB = C_IMG + P            # 231

# push math via Derivative_Erf(x) = (2/sqrt(pi)) * exp(-x^2):
#   arg[p,j] = mean_j + BIGK*(1-valid_j) - mean_p
# valid j != p: exp(-(mean_j-mean_p)^2); invalid j: exp(-(~BIGK)^2) ~ 0;
# diagonal (valid p): derf(0) = 2/sqrt(pi), removed per image via
# psF1 - (2/sqrt(pi))*nt before scaling by sqrt(pi)/2.
SQPI_4 = 0.44311346272637900682   # sqrt(pi)/4

f32 = mybir.dt.float32
bf16 = mybir.dt.bfloat16
i16 = mybir.dt.int16
Alu = mybir.AluOpType
Act = mybir.ActivationFunctionType
AxX = mybir.AxisListType.X


def build_nc() -> bass.Bass:
    nc = bass.Bass()
    tags_d = nc.declare_dram_parameter("tags", [NLOC, NROWS, ROW], f32,
                                       isOutput=False)
    idx_d = nc.declare_dram_parameter("idx", [P, NLOC * NI // 16], i16,
                                      isOutput=False)
    blob_d = nc.declare_dram_parameter("blob", [P, BLOB], f32, isOutput=False)
    oh_d = nc.declare_dram_parameter("oh", [P, CELLS * ROW], bf16,
                                     isOutput=False)
    out_d = nc.declare_dram_parameter("out", [NLOC, 2], f32, isOutput=True)

    with tile.TileContext(nc) as tc:
        with ExitStack() as ctx:
            _body(ctx, tc, nc, tags_d, idx_d[:], blob_d[:], oh_d[:], out_d[:])
    _split_multi_waits(nc, max_waits=1)
    return nc


def _split_multi_waits(nc, max_waits=1):
    """Walrus codegen rejects instructions with too many sync-wait commands
    ("Too many sync wait commands", CoreV3GenImpl::setupSyncWait). Tile's
    kernel-tail drain waits on every live semaphore. Split the excess waits
    onto same-engine nops inserted immediately before the offending
    instruction — identical semantics, one wait per instruction."""
    import bass_rust
    fn = nc.m.functions[0]
    for bb in fn.blocks:
        changed = True
        while changed:
            changed = False
            for inst in list(bb.instructions):
                si = inst.sync_info
                if si is None or not si.on_wait or len(si.on_wait) <= max_waits:
                    continue
                waits = list(si.on_wait)
                keep, rest = waits[:max_waits], waits[max_waits:]
                nops = []
                for i in range(0, len(rest), max_waits):
                    nop_inst = nc.engines[inst.engine].nop().ins
                    nop_inst.sync_info = bass_rust.SyncInfo(
                        on_wait=rest[i:i + max_waits], on_update=[])
                    nops.append(nop_inst)
                inst.sync_info = bass_rust.SyncInfo(
                    on_wait=keep, on_update=list(si.on_update))
                for b2 in fn.blocks:
                    lst = b2.instructions
                    for i in range(len(lst) - 1, -1, -1):
                        if any(lst[i].name == n.name for n in nops):
                            del lst[i]
                idx = next(i for i, x in enumerate(bb.instructions)
                           if x.name == inst.name)
                for j, n in enumerate(nops):
                    bb.instructions.insert(idx + j, n)
                changed = True
                break


def _body(ctx, tc, nc, tags_d, idx, blob, oh, out):
    pool = ctx.enter_context(tc.tile_pool(name="main", bufs=1))
    psum = ctx.enter_context(tc.tile_pool(name="psum", bufs=1, space="PSUM"))

    # DMAGatherAnt lives in the 'mlp' Q7 library; load it before any gather.
    # The plain-Bass path never assembles the pseudo's instruction bytes
    # (only Bacc's codegen does), which walrus rejects as "ISA wrong length".
    # Assemble the NEURON_ISA_TPB_PSEUDO_LIBRARY_RELOAD_INDEX_STRUCT bytes
    # ourselves so walrus emits a runtime-interpreted PSEUDO_INST.
    import concourse.bass_isa as bass_isa
    reload_inst = nc.gpsimd.load_library(library_config.mlp)
    pseudo_op = nc.isa.get_enum("NEURON_ISA_TPB_PSEUDO_OPCODE")
    instr_bytes, _ = bass_isa.isa_struct(
        nc.isa, nc.isa.Opcode.NEURON_ISA_TPB_OPCODE_PSEUDO_INST,
        {"pseudo_opcode":
         pseudo_op.NEURON_ISA_TPB_PSEUDO_OPCODE_PSEUDO_LIBRARY_RELOAD_INDEX.value,
         "lib_index": library_config.mlp.index},
        struct_name="NEURON_ISA_TPB_PSEUDO_LIBRARY_RELOAD_INDEX_STRUCT")
    reload_inst.ins.instr = instr_bytes

    # ---- input DMAs ---------------------------------------------------------
    # chunk-0 idxs ride alone so gather 0 can start ~200ns earlier; the
    # remaining chunks' idxs arrive well before their gathers issue.
    NIC = NI // 16
    sb_idx = pool.tile([P, NLOC * NIC], i16)
    nc.sync.dma_start(out=sb_idx[:, 0:NIC], in_=idx[:, 0:NIC])
    nc.sync.dma_start(out=sb_idx[:, NIC:], in_=idx[:, NIC:])
    sb_blob = pool.tile([P, BLOB], f32)
    nc.sync.dma_start(out=sb_blob, in_=blob)
    sb_oh = pool.tile([P, CELLS, ROW], bf16)
    nc.sync.dma_start(out=sb_oh, in_=oh)

    vis_scat = sb_blob[:, C_VIS:C_VIS + CELLS]
    visp = sb_blob[:, C_VISP:C_VISP + K]
    pers30 = sb_blob[:, C_PERS:C_PERS + M]
    ind32 = sb_blob[:, C_IND:C_IND + MP]
    seg = sb_blob[:, C_SEG:C_SEG + NLOC]
    imgblk = sb_blob[:, C_IMG:C_IMG + P]

    # ---- per-image gathers (Pool SWDGE), issued ASAP ------------------------
    rows = []
    for c in range(NLOC):
        rows_c = pool.tile([P, SLOTS, ROW], f32, tag=f"rows{c}")
        nc.gpsimd.dma_gather(
            out_ap=rows_c[:],
            in_ap=tags_d[c],
            idxs_ap=sb_idx[:, c * (NI // 16):(c + 1) * (NI // 16)],
            num_idxs=NI,
            num_idxs_reg=NI,
            elem_size=ROW,
        )
        rows.append(rows_c)

    # ---- visibility-derived scalars (overlap the gathers) -------------------
    visf = pool.tile([P, CELLS], f32)      # scattered-layout 0/1 mask
    nc.vector.tensor_scalar(out=visf, in0=vis_scat, scalar1=0.0, scalar2=None,
                            op0=Alu.is_gt)
    visfp = pool.tile([P, K], f32)         # person-layout 0/1 mask
    nc.vector.tensor_scalar(out=visfp, in0=visp, scalar1=0.0, scalar2=None,
                            op0=Alu.is_gt)
    cnt = pool.tile([P, 1], f32)
    nc.vector.reduce_sum(out=cnt, in_=visfp, axis=AxX)
    valid = pool.tile([P, 1], f32)
    nc.vector.tensor_scalar(out=valid, in0=cnt, scalar1=1.0, scalar2=None,
                            op0=Alu.min)
    safecnt = pool.tile([P, 1], f32)
    nc.vector.tensor_scalar(out=safecnt, in0=cnt, scalar1=1.0, scalar2=None,
                            op0=Alu.max)
    rc = pool.tile([P, 1], f32)
    nc.vector.reciprocal(out=rc, in_=safecnt)
    sm = pool.tile([P, 1], f32)            # BIGK*(1-valid)
    nc.vector.tensor_scalar(out=sm, in0=valid, scalar1=-BIG, scalar2=BIG,
                            op0=Alu.mult, op1=Alu.add)
    segv = pool.tile([P, NLOC], f32)       # SEG gated by person validity
    nc.vector.tensor_scalar(out=segv, in0=seg, scalar1=valid, scalar2=None,
                            op0=Alu.mult)

    # per-image tag counts + push denominators (all early, off critical path)
    psNT = psum.tile([NLOC, 1], f32, tag="psNT")
    nc.tensor.matmul(out=psNT[:], lhsT=seg, rhs=valid, start=True, stop=True)
    nt = pool.tile([NLOC, 1], f32)
    nc.vector.tensor_copy(out=nt, in_=psNT)
    sant = pool.tile([NLOC, 1], f32)
    nc.vector.tensor_scalar(out=sant, in0=nt, scalar1=1.0, scalar2=None,
                            op0=Alu.max)
    rnt = pool.tile([NLOC, 1], f32)
    nc.vector.reciprocal(out=rnt, in_=sant)
    npr = pool.tile([NLOC, 1], f32)
    nc.vector.scalar_tensor_tensor(out=npr, in0=nt, scalar=-1.0, in1=nt,
                                   op0=Alu.add, op1=Alu.mult)  # (nt-1)*nt
    gate = pool.tile([NLOC, 1], f32)
    nc.vector.tensor_scalar(out=gate, in0=npr, scalar1=0.0, scalar2=None,
                            op0=Alu.is_gt)
    sanp = pool.tile([NLOC, 1], f32)
    nc.vector.tensor_scalar(out=sanp, in0=npr, scalar1=1.0, scalar2=None,
                            op0=Alu.max)
    rnp = pool.tile([NLOC, 1], f32)
    nc.vector.reciprocal(out=rnp, in_=sanp)
    ga = pool.tile([NLOC, 1], f32)         # sqrt(pi)/4 * gate * rnp
    nc.vector.scalar_tensor_tensor(out=ga, in0=gate, scalar=SQPI_4, in1=rnp,
                                   op0=Alu.mult, op1=Alu.mult)
    gb0 = pool.tile([NLOC, 1], f32)
    nc.vector.scalar_tensor_tensor(out=gb0, in0=rnp, scalar=0.5, in1=nt,
                                   op0=Alu.mult, op1=Alu.mult)
    gb = pool.tile([NLOC, 1], f32)         # 0.5 * nt * gate * rnp
    nc.vector.tensor_tensor(out=gb, in0=gb0, in1=gate, op=Alu.mult)

    # ---- per-chunk selection + cell stats + person-stats matmul -------------
    A = pool.tile([P, CELLS], f32)
    GV = pool.tile([P, 2, CELLS], f32)
    GVR = pool.tile([P, NLOC, 2], f32)     # per-chunk slot-reduced (S1,S2)
    psS = psum.tile([P, 2], f32, tag="psS")

    for c in range(NLOC):
        cs = slice(c * SLOTS, (c + 1) * SLOTS)
        sel_c = pool.tile([P, SLOTS, ROW], f32, tag=f"sel{c}")
        nc.vector.tensor_tensor(out=sel_c, in0=rows[c][:],
                                in1=sb_oh[:, cs, :], op=Alu.mult)
        nc.vector.reduce_sum(out=A[:, cs], in_=sel_c[:], axis=AxX)
        nc.vector.tensor_tensor(out=GV[:, 0, cs], in0=A[:, cs],
                                in1=visf[:, cs], op=Alu.mult)
        nc.vector.tensor_tensor(out=GV[:, 1, cs], in0=GV[:, 0, cs],
                                in1=A[:, cs], op=Alu.mult)
        nc.vector.reduce_sum(out=GVR[:, c, :], in_=GV[:, :, cs], axis=AxX)
        nc.tensor.matmul(out=psS[c * MP:(c + 1) * MP, :], lhsT=ind32,
                         rhs=GVR[:, c, :], start=True, stop=True,
                         tile_position=(0, c * MP))

    # ---- per-person mean / push arg / pull ----------------------------------
    smm = pool.tile([P, 1], f32)           # mean + BIGK*(1-valid)
    nc.vector.scalar_tensor_tensor(out=smm, in0=psS[:, 0:1], scalar=rc,
                                   in1=sm, op0=Alu.mult, op1=Alu.add)
    mrhs = pool.tile([P, M], f32)
    nc.vector.tensor_scalar(out=mrhs, in0=pers30, scalar1=smm, scalar2=None,
                            op0=Alu.mult)
    psRep = psum.tile([P, M], f32, tag="psRep")
    nc.tensor.matmul(out=psRep[:], lhsT=imgblk, rhs=mrhs[:],
                     start=True, stop=True)
    negmean = pool.tile([P, 1], f32)
    nc.vector.tensor_scalar(out=negmean, in0=psS[:, 0:1], scalar1=rc,
                            scalar2=-1.0, op0=Alu.mult, op1=Alu.mult)
    mean2 = pool.tile([P, 1], f32)
    nc.vector.tensor_tensor(out=mean2, in0=negmean, in1=negmean, op=Alu.mult)
    p1 = pool.tile([P, 1], f32)
    nc.vector.scalar_tensor_tensor(out=p1, in0=psS[:, 1:2], scalar=rc,
                                   in1=mean2, op0=Alu.mult, op1=Alu.subtract)
    pullred = pool.tile([P, 1], f32)
    nc.vector.tensor_tensor(out=pullred, in0=p1, in1=valid, op=Alu.mult)

    # derf(arg) = (2/sqrt(pi)) exp(-arg^2); row sum in the ACT accumulator
    pe = pool.tile([P, M], f32)
    rowsumv = pool.tile([P, 1], f32)
    nc.scalar.activation(out=pe, in_=psRep[:], func=Act.Derivative_Erf,
                         bias=negmean, accum_out=rowsumv)

    # ---- per-image segment sums + finals ------------------------------------
    psF = psum.tile([NLOC, 2], f32, tag="psF")
    nc.tensor.matmul(out=psF[:, 0:1], lhsT=seg, rhs=pullred[:],
                     start=True, stop=True)
    nc.tensor.matmul(out=psF[:, 1:2], lhsT=segv, rhs=rowsumv[:],
                     start=True, stop=True)
    f42 = pool.tile([NLOC, 2], f32)
    nc.vector.tensor_scalar(out=f42[:, 0:1], in0=psF[:, 0:1], scalar1=rnt,
                            scalar2=None, op0=Alu.mult)
    nc.vector.scalar_tensor_tensor(out=f42[:, 1:2], in0=psF[:, 1:2],
                                   scalar=ga, in1=gb, op0=Alu.mult,
                                   op1=Alu.subtract)
    nc.sync.dma_start(out=out, in_=f42)


# ---------------------------------------------------------------------------
# host side
# ---------------------------------------------------------------------------

def _build_consts():
    p = np.arange(P)
    j30 = np.arange(M)
    pers30 = ((p[:, None] % MP) == j30[None, :]).astype(np.float32)
    ind32 = ((p[:, None] // 4) == np.arange(MP)[None, :]).astype(np.float32)
    seg = (((p[:, None] // MP) == np.arange(NLOC)[None, :])
           & ((p[:, None] % MP) < M)).astype(np.float32)
    imgblk = ((p[:, None] // MP) == (p[None, :] // MP)).astype(np.float32)
    return np.concatenate([pers30, ind32, seg, imgblk],
                          axis=1).astype(np.float32)  # [128, 194]


_CONSTS = _build_consts()

# cell mapping: joint k of person m -> partition 4m + k//5, slot k%5
_MM, _KK = np.meshgrid(np.arange(M), np.arange(K), indexing="ij")
_CELL_P = (4 * _MM + _KK // SLOTS).ravel()          # [510]
_CELL_S = (_KK % SLOTS).ravel()                     # [510]
_IDX_I = (_CELL_S * P + _CELL_P)                    # gather idx position i


def make_in_maps(tags: np.ndarray, joints: np.ndarray):
    tags = np.ascontiguousarray(np.asarray(tags, dtype=np.float32))
    jt = np.asarray(joints)
    loc = np.clip(jt[..., 0], 0, KHW - 1).astype(np.int64)   # [N, M, K]
    visraw = jt[..., 1].astype(np.float32)                   # [N, M, K]
    row = (loc // ROW).astype(np.int16)                      # [N, M, K]
    sub = (loc % ROW).astype(np.int64)                       # [N, M, K]

    # gather idx arrays [N, 640] -> SBUF wrap [N, 128, 40]
    idx_all = np.zeros((N, NI), dtype=np.int16)
    idx_all[:, _IDX_I] = row.reshape(N, M * K)
    idx_sb = np.tile(idx_all.reshape(N, NI // 16, 16).transpose(0, 2, 1),
                     (1, 8, 1))                              # [N, 128, 40]

    # selection one-hot [N, 128, SLOTS*ROW]
    onehot = np.zeros((N, P, SLOTS * ROW), dtype=np.float32)
    cellflat = (_CELL_S * ROW)[None, :] + sub.reshape(N, M * K)  # [N, 510]
    onehot[np.arange(N)[:, None], _CELL_P[None, :], cellflat] = 1.0

    # scattered vis [N, 128, SLOTS]
    vis_scat = np.zeros((N, P, SLOTS), dtype=np.float32)
    vis_scat[np.arange(N)[:, None], _CELL_P[None, :],
             _CELL_S[None, :]] = visraw.reshape(N, M * K)

    # person-layout vis [N, 32, K]
    visp = np.zeros((N, MP, K), dtype=np.float32)
    visp[:, :M, :] = visraw

    in_maps = []
    for c in range(NCORES):
        sl = slice(c * NLOC, (c + 1) * NLOC)
        blob = np.concatenate([
            vis_scat[sl].transpose(1, 0, 2).reshape(P, CELLS),
            visp[sl].reshape(P, K),
            _CONSTS,
        ], axis=1)
        in_maps.append({
            "tags": tags[sl].reshape(NLOC, NROWS, ROW),
            "idx": np.ascontiguousarray(
                np.concatenate(idx_sb[sl], axis=1)),     # [128, 160]
            "blob": np.ascontiguousarray(blob),
            "oh": np.ascontiguousarray(
                onehot[sl].transpose(1, 0, 2).reshape(P, CELLS * ROW)
                .astype(ml_dtypes.bfloat16)),
        })
    return in_maps


_NC_CACHE = None


def _get_nc():
    global _NC_CACHE
    if _NC_CACHE is None:
        _NC_CACHE = build_nc()
    return _NC_CACHE


def kernel(tags: np.ndarray, joints: np.ndarray, _bench_results=None):
    nc = _get_nc()
    in_maps = make_in_maps(tags, joints)
    res = run_bass_kernel_spmd(nc, in_maps, core_ids=list(range(NCORES)))
    if _bench_results is not None:
        _bench_results.append(res)
    per_image = np.concatenate([r["out"] for r in res.results], axis=0)
    pull_loss = np.float32(per_image[:, 0].mean(dtype=np.float64))
    push_loss = np.float32(per_image[:, 1].mean(dtype=np.float64))
    return pull_loss, push_loss


# revision 21
# speedup vs baseline: 1.9143x; 1.0439x over previous
"""Associative-embedding loss on 8 Trainium2 NeuronCores.

Data-parallel over batch N=32: each of the 8 cores handles 4 images.

Gather strategy: the per-image flat tag array (1114112 f32) is viewed as
17408 rows of 64 f32 (256B).  One SWDGE dma_gather per image fetches the
row containing each needed element (int16 row indices fit: 17408<32768).
Row indices and a {0,1} selection one-hot for the element-within-row are
precomputed on the host (pure address arithmetic).  Each image's
30 persons x 17 joints are spread over all 128 partitions (person
m <-> partitions 4m..4m+3, 5 slots each, 20 cells >= 17) so a single
gather delivers every partition's work.

On-chip pipeline per chunk (image): rows*onehot -> reduce(64) -> A[p,s];
gsv=A*vis, gsv2=gsv*A; a K=128 matmul against the constant indicator
IND32[p,m]=[p//4==m] reduces cells to per-person (sum v*g, sum v*g^2).
pull uses the variance identity; push replicates per-image (mean, valid)
rows to every partition with one K=128 matmul against a block-diagonal
constant, then exp(-(m_p-m_j)^2 - BIG*(1-v_j) - BIG*diag) on ACT with a
fused row-sum accumulator.  Per-image segment sums via one PE matmul.
All visibility-derived scalars (cnt, 1/cnt, valid, per-image pair counts)
are computed while the gathers are still in flight.
"""

import numpy as np
import ml_dtypes
from contextlib import ExitStack

import concourse.bass as bass
import concourse.tile as tile
from concourse import library_config, mybir
from concourse.bass_utils import run_bass_kernel_spmd

# Problem constants (hardcoded per contract).
N, K, H, W, M = 32, 17, 256, 256, 30
NCORES = 8
NLOC = N // NCORES          # images per core
KHW = K * H * W             # 1114112 flat tag elements per image
MP = 32                     # padded persons per image
P = 128                     # SBUF partitions
BIG = 30.0                  # exp(-BIG) ~ 9e-14: kills masked columns

ROW = 64                    # gather row width (f32 elems, 256B)
NROWS = KHW // ROW          # 17408 rows per image (< int16 range)
SLOTS = 4                   # gather slots per partition per image
NI = SLOTS * P              # 512 idxs per per-image gather
CELLS = NLOC * SLOTS        # 16 scattered cells per partition

# blob column offsets
C_VIS = 0                   # [128, 20] raw vis values, scattered layout
C_VISP = C_VIS + CELLS      # [128, 17] raw vis values, person layout
C_PERS = C_VISP + K         # [128, 30] PERS30[p, j] = (p%32 == j)
C_IND = C_PERS + M          # [128, 4*32] IND4[p, s*32+m] = (cell (p,s) of person m)
C_SEG = C_IND + SLOTS * MP  # [128, 4]  SEG[p, i] = (p//32==i and p%32<30)
C_IMG = C_SEG + NLOC        # [128, 128] IMGBLK[q, p] = (q//32 == p//32)
BLOB = C_IMG + P            # 327

# push math via Derivative_Erf(x) = (2/sqrt(pi)) * exp(-x^2):
#   arg[p,j] = mean_j + BIGK*(1-valid_j) - mean_p
# valid j != p: exp(-(mean_j-mean_p)^2); invalid j: exp(-(~BIGK)^2) ~ 0;
# diagonal (valid p): derf(0) = 2/sqrt(pi), removed per image via
# psF1 - (2/sqrt(pi))*nt before scaling by sqrt(pi)/2.
SQPI_4 = 0.44311346272637900682   # sqrt(pi)/4

f32 = mybir.dt.float32
bf16 = mybir.dt.bfloat16
i16 = mybir.dt.int16
Alu = mybir.AluOpType
Act = mybir.ActivationFunctionType
AxX = mybir.AxisListType.X


def build_nc() -> bass.Bass:
    nc = bass.Bass()
    tags_d = nc.declare_dram_parameter("tags", [NLOC, NROWS, ROW], f32,
                                       isOutput=False)
    idx_d = nc.declare_dram_parameter("idx", [P, NLOC * NI // 16], i16,
                                      isOutput=False)
    blob_d = nc.declare_dram_parameter("blob", [P, BLOB], f32, isOutput=False)
    oh_d = nc.declare_dram_parameter("oh", [P, CELLS * ROW], bf16,
                                     isOutput=False)
    out_d = nc.declare_dram_parameter("out", [NLOC, 2], f32, isOutput=True)

    with tile.TileContext(nc) as tc:
        with ExitStack() as ctx:
            _body(ctx, tc, nc, tags_d, idx_d[:], blob_d[:], oh_d[:], out_d[:])
    _split_multi_waits(nc, max_waits=1)
    return nc


def _split_multi_waits(nc, max_waits=1):
    """Walrus codegen rejects instructions with too many sync-wait commands
    ("Too many sync wait commands", CoreV3GenImpl::setupSyncWait). Tile's
    kernel-tail drain waits on every live semaphore. Split the excess waits
    onto same-engine nops inserted immediately before the offending
    instruction — identical semantics, one wait per instruction."""
    import bass_rust
    fn = nc.m.functions[0]
    for bb in fn.blocks:
        changed = True
        while changed:
            changed = False
            for inst in list(bb.instructions):
                si = inst.sync_info
                if si is None or not si.on_wait or len(si.on_wait) <= max_waits:
                    continue
                waits = list(si.on_wait)
                keep, rest = waits[:max_waits], waits[max_waits:]
                nops = []
                for i in range(0, len(rest), max_waits):
                    nop_inst = nc.engines[inst.engine].nop().ins
                    nop_inst.sync_info = bass_rust.SyncInfo(
                        on_wait=rest[i:i + max_waits], on_update=[])
                    nops.append(nop_inst)
                inst.sync_info = bass_rust.SyncInfo(
                    on_wait=keep, on_update=list(si.on_update))
                for b2 in fn.blocks:
                    lst = b2.instructions
                    for i in range(len(lst) - 1, -1, -1):
                        if any(lst[i].name == n.name for n in nops):
                            del lst[i]
                idx = next(i for i, x in enumerate(bb.instructions)
                           if x.name == inst.name)
                for j, n in enumerate(nops):
                    bb.instructions.insert(idx + j, n)
                changed = True
                break


def _body(ctx, tc, nc, tags_d, idx, blob, oh, out):
    pool = ctx.enter_context(tc.tile_pool(name="main", bufs=1))
    psum = ctx.enter_context(tc.tile_pool(name="psum", bufs=1, space="PSUM"))

    # DMAGatherAnt lives in the 'mlp' Q7 library; load it before any gather.
    # The plain-Bass path never assembles the pseudo's instruction bytes
    # (only Bacc's codegen does), which walrus rejects as "ISA wrong length".
    # Assemble the NEURON_ISA_TPB_PSEUDO_LIBRARY_RELOAD_INDEX_STRUCT bytes
    # ourselves so walrus emits a runtime-interpreted PSEUDO_INST.
    import concourse.bass_isa as bass_isa
    reload_inst = nc.gpsimd.load_library(library_config.mlp)
    pseudo_op = nc.isa.get_enum("NEURON_ISA_TPB_PSEUDO_OPCODE")
    instr_bytes, _ = bass_isa.isa_struct(
        nc.isa, nc.isa.Opcode.NEURON_ISA_TPB_OPCODE_PSEUDO_INST,
        {"pseudo_opcode":
         pseudo_op.NEURON_ISA_TPB_PSEUDO_OPCODE_PSEUDO_LIBRARY_RELOAD_INDEX.value,
         "lib_index": library_config.mlp.index},
        struct_name="NEURON_ISA_TPB_PSEUDO_LIBRARY_RELOAD_INDEX_STRUCT")
    reload_inst.ins.instr = instr_bytes

    # ---- input DMAs ---------------------------------------------------------
    # chunk-0 idxs ride alone so gather 0 can start ~200ns earlier; the
    # remaining chunks' idxs arrive well before their gathers issue.
    NIC = NI // 16
    sb_idx = pool.tile([P, NLOC * NIC], i16)
    nc.sync.dma_start(out=sb_idx[:, 0:NIC], in_=idx[:, 0:NIC])
    nc.sync.dma_start(out=sb_idx[:, NIC:], in_=idx[:, NIC:])
    sb_blob = pool.tile([P, BLOB], f32)
    nc.sync.dma_start(out=sb_blob, in_=blob)
    sb_oh = pool.tile([P, CELLS, ROW], bf16)
    nc.sync.dma_start(out=sb_oh, in_=oh)

    vis_scat = sb_blob[:, C_VIS:C_VIS + CELLS]
    visp = sb_blob[:, C_VISP:C_VISP + K]
    pers30 = sb_blob[:, C_PERS:C_PERS + M]
    ind4 = sb_blob[:, C_IND:C_IND + SLOTS * MP]
    seg = sb_blob[:, C_SEG:C_SEG + NLOC]
    imgblk = sb_blob[:, C_IMG:C_IMG + P]

    # ---- per-image gathers (Pool SWDGE), issued ASAP ------------------------
    rows = []
    for c in range(NLOC):
        rows_c = pool.tile([P, SLOTS, ROW], f32, tag=f"rows{c}")
        nc.gpsimd.dma_gather(
            out_ap=rows_c[:],
            in_ap=tags_d[c],
            idxs_ap=sb_idx[:, c * (NI // 16):(c + 1) * (NI // 16)],
            num_idxs=NI,
            num_idxs_reg=NI,
            elem_size=ROW,
        )
        rows.append(rows_c)

    # ---- visibility-derived scalars (overlap the gathers) -------------------
    visf = pool.tile([P, CELLS], f32)      # scattered-layout 0/1 mask
    nc.vector.tensor_scalar(out=visf, in0=vis_scat, scalar1=0.0, scalar2=None,
                            op0=Alu.is_gt)
    visfp = pool.tile([P, K], f32)         # person-layout 0/1 mask
    nc.vector.tensor_scalar(out=visfp, in0=visp, scalar1=0.0, scalar2=None,
                            op0=Alu.is_gt)
    cnt = pool.tile([P, 1], f32)
    nc.vector.reduce_sum(out=cnt, in_=visfp, axis=AxX)
    valid = pool.tile([P, 1], f32)
    nc.vector.tensor_scalar(out=valid, in0=cnt, scalar1=1.0, scalar2=None,
                            op0=Alu.min)
    safecnt = pool.tile([P, 1], f32)
    nc.vector.tensor_scalar(out=safecnt, in0=cnt, scalar1=1.0, scalar2=None,
                            op0=Alu.max)
    rc = pool.tile([P, 1], f32)
    nc.vector.reciprocal(out=rc, in_=safecnt)
    sm = pool.tile([P, 1], f32)            # BIGK*(1-valid)
    nc.vector.tensor_scalar(out=sm, in0=valid, scalar1=-BIG, scalar2=BIG,
                            op0=Alu.mult, op1=Alu.add)
    segv = pool.tile([P, NLOC], f32)       # SEG gated by person validity
    nc.vector.tensor_scalar(out=segv, in0=seg, scalar1=valid, scalar2=None,
                            op0=Alu.mult)

    # per-image tag counts + push denominators (all early, off critical path)
    psNT = psum.tile([NLOC, 1], f32, tag="psNT")
    nc.tensor.matmul(out=psNT[:], lhsT=seg, rhs=valid, start=True, stop=True)
    nt = pool.tile([NLOC, 1], f32)
    nc.vector.tensor_copy(out=nt, in_=psNT)
    sant = pool.tile([NLOC, 1], f32)
    nc.vector.tensor_scalar(out=sant, in0=nt, scalar1=1.0, scalar2=None,
                            op0=Alu.max)
    rnt = pool.tile([NLOC, 1], f32)
    nc.vector.reciprocal(out=rnt, in_=sant)
    npr = pool.tile([NLOC, 1], f32)
    nc.vector.scalar_tensor_tensor(out=npr, in0=nt, scalar=-1.0, in1=nt,
                                   op0=Alu.add, op1=Alu.mult)  # (nt-1)*nt
    gate = pool.tile([NLOC, 1], f32)
    nc.vector.tensor_scalar(out=gate, in0=npr, scalar1=0.0, scalar2=None,
                            op0=Alu.is_gt)
    sanp = pool.tile([NLOC, 1], f32)
    nc.vector.tensor_scalar(out=sanp, in0=npr, scalar1=1.0, scalar2=None,
                            op0=Alu.max)
    rnp = pool.tile([NLOC, 1], f32)
    nc.vector.reciprocal(out=rnp, in_=sanp)
    ga = pool.tile([NLOC, 1], f32)         # sqrt(pi)/4 * gate * rnp
    nc.vector.scalar_tensor_tensor(out=ga, in0=gate, scalar=SQPI_4, in1=rnp,
                                   op0=Alu.mult, op1=Alu.mult)
    gb0 = pool.tile([NLOC, 1], f32)
    nc.vector.scalar_tensor_tensor(out=gb0, in0=rnp, scalar=0.5, in1=nt,
                                   op0=Alu.mult, op1=Alu.mult)
    gb = pool.tile([NLOC, 1], f32)         # 0.5 * nt * gate * rnp
    nc.vector.tensor_tensor(out=gb, in0=gb0, in1=gate, op=Alu.mult)

    # ---- per-chunk selection + cell stats + person-stats matmul -------------
    A = pool.tile([P, CELLS], f32)
    GV = pool.tile([P, 2, CELLS], f32)
    psS = psum.tile([P, 2], f32, tag="psS")

    for c in range(NLOC):
        cs = slice(c * SLOTS, (c + 1) * SLOTS)
        sel_c = pool.tile([P, SLOTS, ROW], f32, tag=f"sel{c}")
        nc.vector.tensor_tensor(out=sel_c, in0=rows[c][:],
                                in1=sb_oh[:, cs, :], op=Alu.mult)
        nc.vector.reduce_sum(out=A[:, cs], in_=sel_c[:], axis=AxX)
        nc.vector.tensor_tensor(out=GV[:, 0, cs], in0=A[:, cs],
                                in1=visf[:, cs], op=Alu.mult)
        nc.vector.tensor_tensor(out=GV[:, 1, cs], in0=GV[:, 0, cs],
                                in1=A[:, cs], op=Alu.mult)
        for s in range(SLOTS):
            nc.tensor.matmul(out=psS[c * MP:(c + 1) * MP, :],
                             lhsT=ind4[:, s * MP:(s + 1) * MP],
                             rhs=GV[:, :, c * SLOTS + s],
                             start=(s == 0), stop=(s == SLOTS - 1),
                             tile_position=(0, c * MP))

    # ---- per-person mean / push arg / pull ----------------------------------
    smm = pool.tile([P, 1], f32)           # mean + BIGK*(1-valid)
    nc.vector.scalar_tensor_tensor(out=smm, in0=psS[:, 0:1], scalar=rc,
                                   in1=sm, op0=Alu.mult, op1=Alu.add)
    mrhs = pool.tile([P, M], f32)
    nc.vector.tensor_scalar(out=mrhs, in0=pers30, scalar1=smm, scalar2=None,
                            op0=Alu.mult)
    psRep = psum.tile([P, M], f32, tag="psRep")
    nc.tensor.matmul(out=psRep[:], lhsT=imgblk, rhs=mrhs[:],
                     start=True, stop=True)
    negmean = pool.tile([P, 1], f32)
    nc.vector.tensor_scalar(out=negmean, in0=psS[:, 0:1], scalar1=rc,
                            scalar2=-1.0, op0=Alu.mult, op1=Alu.mult)
    mean2 = pool.tile([P, 1], f32)
    nc.vector.tensor_tensor(out=mean2, in0=negmean, in1=negmean, op=Alu.mult)
    p1 = pool.tile([P, 1], f32)
    nc.vector.scalar_tensor_tensor(out=p1, in0=psS[:, 1:2], scalar=rc,
                                   in1=mean2, op0=Alu.mult, op1=Alu.subtract)
    pullred = pool.tile([P, 1], f32)
    nc.vector.tensor_tensor(out=pullred, in0=p1, in1=valid, op=Alu.mult)

    # derf(arg) = (2/sqrt(pi)) exp(-arg^2); row sum in the ACT accumulator
    pe = pool.tile([P, M], f32)
    rowsumv = pool.tile([P, 1], f32)
    nc.scalar.activation(out=pe, in_=psRep[:], func=Act.Derivative_Erf,
                         bias=negmean, accum_out=rowsumv)

    # ---- per-image segment sums + finals ------------------------------------
    psF = psum.tile([NLOC, 2], f32, tag="psF")
    nc.tensor.matmul(out=psF[:, 0:1], lhsT=seg, rhs=pullred[:],
                     start=True, stop=True)
    nc.tensor.matmul(out=psF[:, 1:2], lhsT=segv, rhs=rowsumv[:],
                     start=True, stop=True)
    f42 = pool.tile([NLOC, 2], f32)
    nc.vector.tensor_scalar(out=f42[:, 0:1], in0=psF[:, 0:1], scalar1=rnt,
                            scalar2=None, op0=Alu.mult)
    nc.vector.scalar_tensor_tensor(out=f42[:, 1:2], in0=psF[:, 1:2],
                                   scalar=ga, in1=gb, op0=Alu.mult,
                                   op1=Alu.subtract)
    nc.sync.dma_start(out=out, in_=f42)


# ---------------------------------------------------------------------------
# host side
# ---------------------------------------------------------------------------

def _build_consts():
    p = np.arange(P)
    j30 = np.arange(M)
    pers30 = ((p[:, None] % MP) == j30[None, :]).astype(np.float32)
    # IND4[p, s*32+m] = 1 iff gather cell i = s*128+p is a real cell of
    # person m (column-major packing: cell i == m*K + k for i < M*K)
    i_grid = np.arange(SLOTS)[:, None] * P + p[None, :]      # [4, 128]
    ind4 = np.zeros((P, SLOTS, MP), dtype=np.float32)
    for s in range(SLOTS):
        mcell = i_grid[s] // K
        real = i_grid[s] < M * K
        ind4[real, s, mcell[real]] = 1.0
    seg = (((p[:, None] // MP) == np.arange(NLOC)[None, :])
           & ((p[:, None] % MP) < M)).astype(np.float32)
    imgblk = ((p[:, None] // MP) == (p[None, :] // MP)).astype(np.float32)
    return np.concatenate([pers30, ind4.reshape(P, SLOTS * MP), seg, imgblk],
                          axis=1).astype(np.float32)  # [128, 290]


_CONSTS = _build_consts()

# cell mapping: joint k of person m -> cell i = m*K + k; partition i%128,
# slot i//128 (matches the gather's idx->dest wrap)
_MM, _KK = np.meshgrid(np.arange(M), np.arange(K), indexing="ij")
_CI = (_MM * K + _KK).ravel()                       # [510]
_CELL_P = _CI % P
_CELL_S = _CI // P
_IDX_I = _CI                                        # gather idx position i


def make_in_maps(tags: np.ndarray, joints: np.ndarray):
    tags = np.ascontiguousarray(np.asarray(tags, dtype=np.float32))
    jt = np.asarray(joints)
    loc = np.clip(jt[..., 0], 0, KHW - 1).astype(np.int64)   # [N, M, K]
    visraw = jt[..., 1].astype(np.float32)                   # [N, M, K]
    row = (loc // ROW).astype(np.int16)                      # [N, M, K]
    sub = (loc % ROW).astype(np.int64)                       # [N, M, K]

    # gather idx arrays [N, 640] -> SBUF wrap [N, 128, 40]
    idx_all = np.zeros((N, NI), dtype=np.int16)
    idx_all[:, _IDX_I] = row.reshape(N, M * K)
    idx_sb = np.tile(idx_all.reshape(N, NI // 16, 16).transpose(0, 2, 1),
                     (1, 8, 1))                              # [N, 128, 40]

    # selection one-hot [N, 128, SLOTS*ROW]
    onehot = np.zeros((N, P, SLOTS * ROW), dtype=np.float32)
    cellflat = (_CELL_S * ROW)[None, :] + sub.reshape(N, M * K)  # [N, 510]
    onehot[np.arange(N)[:, None], _CELL_P[None, :], cellflat] = 1.0

    # scattered vis [N, 128, SLOTS]
    vis_scat = np.zeros((N, P, SLOTS), dtype=np.float32)
    vis_scat[np.arange(N)[:, None], _CELL_P[None, :],
             _CELL_S[None, :]] = visraw.reshape(N, M * K)

    # person-layout vis [N, 32, K]
    visp = np.zeros((N, MP, K), dtype=np.float32)
    visp[:, :M, :] = visraw

    in_maps = []
    for c in range(NCORES):
        sl = slice(c * NLOC, (c + 1) * NLOC)
        blob = np.concatenate([
            vis_scat[sl].transpose(1, 0, 2).reshape(P, CELLS),
            visp[sl].reshape(P, K),
            _CONSTS,
        ], axis=1)
        in_maps.append({
            "tags": tags[sl].reshape(NLOC, NROWS, ROW),
            "idx": np.ascontiguousarray(
                np.concatenate(idx_sb[sl], axis=1)),     # [128, 160]
            "blob": np.ascontiguousarray(blob),
            "oh": np.ascontiguousarray(
                onehot[sl].transpose(1, 0, 2).reshape(P, CELLS * ROW)
                .astype(ml_dtypes.bfloat16)),
        })
    return in_maps


_NC_CACHE = None


def _get_nc():
    global _NC_CACHE
    if _NC_CACHE is None:
        _NC_CACHE = build_nc()
    return _NC_CACHE


def kernel(tags: np.ndarray, joints: np.ndarray, _bench_results=None):
    nc = _get_nc()
    in_maps = make_in_maps(tags, joints)
    res = run_bass_kernel_spmd(nc, in_maps, core_ids=list(range(NCORES)))
    if _bench_results is not None:
        _bench_results.append(res)
    per_image = np.concatenate([r["out"] for r in res.results], axis=0)
    pull_loss = np.float32(per_image[:, 0].mean(dtype=np.float64))
    push_loss = np.float32(per_image[:, 1].mean(dtype=np.float64))
    return pull_loss, push_loss
